# revision 19
# baseline (speedup 1.0000x reference)
"""Multi-head self-attention (B=4, T=2048, D=1024, H=16) on 8 trn2 cores.

Sharding: core = b * 2 + g  (b = batch 0..3, g = head-group 0..1, 8 heads each).
Each core computes, for its (batch, 8-head group):
  Q^T,K^T [hd=512, T] and V [T, hd] from x^T (host-pretransposed, bf16),
  per head: E = exp((K^T_h)^T @ Q^T_h / 8)  in [s, t] layout,
  ctx^T/denominator via PV matmul with a ones-row appended to V,
  out-projection partial y_part = ctx^T.T @ w_out[g rows] + b_out (g==0 only).
Host sums the two head-group partials per batch.

All matmuls run in bf16 (1 row/cycle on PE; same rate as f32r but half
the SBUF/DMA); attention tiles are 1024 wide to halve instruction count.
"""

import numpy as np
import ml_dtypes
import concourse.bass as bass
import concourse.bacc as bacc
import concourse.mybir as mybir
import concourse.tile as tile
from concourse.bass_utils import run_bass_kernel_spmd

B, T, D = 4, 2048, 1024
H, DK = 16, 64
G = 2                 # head groups
HPG = H // G          # heads per core = 8
HD = HPG * DK         # 512
NCORES = B * G        # 8
SCALE = 1.0 / float(np.sqrt(DK))

F32 = mybir.dt.float32
BF16 = mybir.dt.bfloat16
Ident = mybir.ActivationFunctionType.Identity
Exp = mybir.ActivationFunctionType.Exp

NCC = D // 128        # 8 contraction chunks
NDT = HD // 128       # 4 d-tiles of Q/K per core
NTT = T // 128        # 16 t-tiles
NSI = T // 128        # 16 s-tiles
NTP = 2               # t-halves of 1024
TPW = T // NTP        # 1024
VW = HPG * (DK + 1)   # 520: V padded with a ones column per head

# knobs
DVE_EXP_SI = {2, 5, 8, 11, 13, 15}   # si whose exp runs on DVE (Schraudolph)

# Schraudolph constants for bf16-bits exp: i16 = round(score*A + Bc)
SCH_A = SCALE * 128.0 / float(np.log(2.0))
SCH_B = 127.0 * 128.0 - 8.0


def build_program():
    nc = bacc.Bacc("TRN2", target_bir_lowering=False, debug=False)

    xt = nc.dram_tensor("xt", [D, T], BF16, kind="ExternalInput").ap()
    wq = nc.dram_tensor("wq", [D, HD], BF16, kind="ExternalInput").ap()
    wk = nc.dram_tensor("wk", [D, HD], BF16, kind="ExternalInput").ap()
    wv = nc.dram_tensor("wv", [D, HD], BF16, kind="ExternalInput").ap()
    bq = nc.dram_tensor("bq", [HD, 1], F32, kind="ExternalInput").ap()
    bk = nc.dram_tensor("bk", [HD, 1], F32, kind="ExternalInput").ap()
    bv = nc.dram_tensor("bv", [128, HD], F32, kind="ExternalInput").ap()
    wo = nc.dram_tensor("wo", [HD, D], BF16, kind="ExternalInput").ap()
    bo = nc.dram_tensor("bo", [128, D], F32, kind="ExternalInput").ap()
    y = nc.dram_tensor("y", [T, D], F32, kind="ExternalOutput").ap()

    with tile.TileContext(nc) as tc:
        with tc.tile_pool(name="persist", bufs=1) as pp, \
             tc.tile_pool(name="ps", bufs=1, space="PSUM") as psp:
            # persistent QKV outputs (bf16)
            qT = [pp.tile([128, T], BF16, name=f"qT{i}", tag=f"qT{i}") for i in range(NDT)]
            kT = [pp.tile([128, T], BF16, name=f"kT{i}", tag=f"kT{i}") for i in range(NDT)]
            vv = [pp.tile([128, VW], BF16, name=f"v{i}", tag=f"v{i}") for i in range(NTT)]
            ctx = [pp.tile([128, T], BF16, name=f"ctx{i}", tag=f"ctx{i}")
                   for i in range(NDT)]

            # out-proj weights, prefetched early (used in phase 3)
            wo_sb = [pp.tile([128, D], BF16, name=f"wo{c}", tag=f"wo{c}")
                     for c in range(NDT)]
            bo_sb = pp.tile([128, D], F32, name="bo_sb", tag="bo_sb")

            # attention working tiles live in p2 (kept open across
            # phases so kb/qd prefetch for the first block can be emitted
            # during phase 1); p1 closes after phase 1 to free xt/w SBUF.
            with tc.tile_pool(name="p2", bufs=1) as p2, \
                 tc.tile_pool(name="dpool", bufs=1, space="DRAM") as dpool:
                # ---------------- Phase 1: QKV projections ----------------
                with tc.tile_pool(name="p1", bufs=1) as p1:
                    ones_f32 = p1.tile([128, 64], F32, name="ones_f32", tag="ones_f32")
                    nc.vector.memset(ones_f32[:], 1.0)
                    # DMA issue order tuned so the first chains can start
                    # ASAP: interleave x half0 chunks with wk (K goes first).
                    xt_half = [[None] * NCC, [None] * NCC]
                    wk_sb = [p1.tile([128, HD], BF16, name=f"wk{c}", tag=f"wk{c}") for c in range(NCC)]
                    wq_sb = [p1.tile([128, HD], BF16, name=f"wq{c}", tag=f"wq{c}") for c in range(NCC)]
                    for c in range(NCC):
                        xh = p1.tile([128, TPW], BF16, name=f"xt_h0_{c}", tag="xt", bufs=12)
                        nc.sync.dma_start(xh[:], xt[c * 128:(c + 1) * 128, 0:TPW])
                        xt_half[0][c] = xh
                        nc.sync.dma_start(wk_sb[c][:], wk[c * 128:(c + 1) * 128, :])
                    bk_sb = [p1.tile([128, 1], F32, name=f"bk{i}", tag=f"bk{i}") for i in range(NDT)]
                    for i in range(NDT):
                        nc.sync.dma_start(bk_sb[i][:], bk[i * 128:(i + 1) * 128, :])
                    for c in range(NCC):
                        nc.sync.dma_start(wq_sb[c][:], wq[c * 128:(c + 1) * 128, :])
                    bq_sb = [p1.tile([128, 1], F32, name=f"bq{i}", tag=f"bq{i}") for i in range(NDT)]
                    for i in range(NDT):
                        nc.sync.dma_start(bq_sb[i][:], bq[i * 128:(i + 1) * 128, :])
                    for c in range(NCC):
                        xh = p1.tile([128, TPW], BF16, name=f"xt_h1_{c}", tag="xt", bufs=12)
                        nc.sync.dma_start(xh[:], xt[c * 128:(c + 1) * 128, TPW:T])
                        xt_half[1][c] = xh
                    wv_sb = [p1.tile([128, HD], BF16, name=f"wv{c}", tag=f"wv{c}") for c in range(NCC)]
                    for c in range(NCC):
                        nc.sync.dma_start(wv_sb[c][:], wv[c * 128:(c + 1) * 128, :])
                    bv_sb = p1.tile([128, HD], F32, name="bv_sb", tag="bv_sb")
                    nc.sync.dma_start(bv_sb[:], bv[:])
                    for c in range(NDT):
                        nc.sync.dma_start(wo_sb[c][:], wo[c * 128:(c + 1) * 128, :])
                    nc.sync.dma_start(bo_sb[:], bo[:])

                    def qk_chain(w_sb, b_sb, out_t, dt_i, half):
                        # psum[d,t] += w[c,d]^T.T @ xt[c,t]  (two N=512 chains
                        # into one [128,1024] slot, drained by one ACT)
                        ps = psp.tile([128, TPW], F32, name="slot", tag="slot", bufs=2)
                        for tci in range(2):
                            for c in range(NCC):
                                nc.tensor.matmul(
                                    ps[:, tci * 512:(tci + 1) * 512],
                                    w_sb[c][:, dt_i * 128:(dt_i + 1) * 128],
                                    xt_half[half][c][:, tci * 512:(tci + 1) * 512],
                                    start=(c == 0), stop=(c == NCC - 1))
                        nc.scalar.activation(
                            out_t[dt_i][:, half * TPW:(half + 1) * TPW],
                            ps[:], Ident, bias=b_sb[dt_i][:])

                    def v_pair(half, tp):
                        # two V t-tiles share one [128,1024] psum slot
                        ps = psp.tile([128, TPW], F32, name="slot", tag="slot", bufs=2)
                        for j in range(2):
                            for c in range(NCC):
                                nc.tensor.matmul(
                                    ps[:, j * 512:(j + 1) * 512],
                                    xt_half[half][c][:, (tp * 2 + j) * 128:(tp * 2 + j + 1) * 128],
                                    wv_sb[c][:],
                                    start=(c == 0), stop=(c == NCC - 1))
                        for j in range(2):
                            tt = half * 8 + tp * 2 + j
                            vt = vv[tt]
                            v3 = vt[:].rearrange("p (h e) -> p h e", e=DK + 1)
                            with nc.allow_low_precision(reason="bf16 for PE"):
                                nc.vector.tensor_add(
                                    v3[:, :, 0:DK],
                                    ps[:, j * 512:(j + 1) * 512].rearrange(
                                        "p (h e) -> p h e", e=DK),
                                    bv_sb[:].rearrange("p (h e) -> p h e", e=DK))
                                nc.vector.tensor_copy(
                                    v3[:, :, DK:DK + 1],
                                    ones_f32[:, 0:HPG].rearrange("p (h e) -> p h e", e=1))

                    # half 0: K first (kb gate), Q dt0 (qd gate), V, rest of Q
                    for dt_i in range(NDT):
                        qk_chain(wk_sb, bk_sb, kT, dt_i, 0)
                    qk_chain(wq_sb, bq_sb, qT, 0, 0)
                    for tp in range(4):
                        v_pair(0, tp)
                    for dt_i in range(1, NDT):
                        qk_chain(wq_sb, bq_sb, qT, dt_i, 0)
                    # half 1: K dt0 + Q dt0 first, then prefetch h0's kb/qd so
                    # the first attention block can start right at phase end.
                    qk_chain(wk_sb, bk_sb, kT, 0, 1)
                    qk_chain(wq_sb, bq_sb, qT, 0, 1)
                    for dt_i in range(1, NDT):
                        qk_chain(wk_sb, bk_sb, kT, dt_i, 1)
                    for tp in range(4):
                        v_pair(1, tp)
                    for dt_i in range(1, NDT):
                        qk_chain(wq_sb, bq_sb, qT, dt_i, 1)

                # ---------------- Phase 2: attention ----------------
                # blocks: h outer, tp (t-half of 1024) inner; kb cached per h.
                blocks = [(h, tp) for tp in range(NTP) for h in range(HPG)]
                prev = None  # (h, tp, e_list, pc)

                def emit_pv(prev, si):
                    h, tp, e_list, pc = prev
                    vsl = vv[si][:, h * (DK + 1):(h + 1) * (DK + 1)]
                    for half in range(2):
                        nc.tensor.matmul(
                            pc[:, half * 512:(half + 1) * 512],
                            vsl, e_list[si][:, half * 512:(half + 1) * 512],
                            start=(si == 0), stop=(si == NSI - 1))

                def emit_norm(prev):
                    # ctx[d,t] = pc[d,t] / pc[64,t]. The [1,1024] denominator
                    # row is bounced through DRAM to spread it over 64
                    # partitions: DVE reciprocal costs 8 cyc/elem/lane, so a
                    # single-partition reciprocal would be 8.5us.
                    h, tp, e_list, pc = prev
                    ti, ro = h // 2, (h % 2) * 64
                    tps = slice(tp * TPW, (tp + 1) * TPW)
                    dsb = p2.tile([1, TPW], F32, name="dn_row", tag="dn_row", bufs=1)
                    nc.vector.tensor_copy(dsb[:], pc[64:65, :])
                    scr1 = dpool.tile([1, TPW], F32, name="dscr", tag="dscr", bufs=2)
                    nc.gpsimd.dma_start(scr1[:], dsb[:])
                    d64 = p2.tile([64, 16], F32, name="d64", tag="d64", bufs=2)
                    nc.gpsimd.dma_start(
                        d64[:], scr1[:].rearrange("o (p f) -> (o p) f", p=64))
                    r64 = p2.tile([64, 16], F32, name="r64", tag="r64", bufs=2)
                    nc.vector.reciprocal(r64[:], d64[:])
                    scr2 = dpool.tile([1, TPW], F32, name="rscr", tag="rscr", bufs=2)
                    nc.gpsimd.dma_start(
                        scr2[:].rearrange("o (p f) -> (o p) f", p=64), r64[:])
                    r = p2.tile([1, TPW], F32, name="r_t", tag="r_t", bufs=1)
                    nc.gpsimd.dma_start(r[:], scr2[:])
                    rb = p2.tile([64, TPW], F32, name="rb_t", tag="rb_t", bufs=2)
                    nc.gpsimd.partition_broadcast(rb[:], r[:])
                    with nc.allow_low_precision(reason="bf16 for PE"):
                        nc.vector.tensor_mul(
                            ctx[ti][ro:ro + 64, tps], pc[0:64, :], rb[:])

                # Scores use the full [128, s-block] kT slice as the
                # stationary (both heads of the pair; K=128 keeps the HAM
                # clock gate open) against a moving Q whose other-head rows
                # are zeroed, so the off-head contribution vanishes. Two
                # persistent buffers per parity: the zero half is written
                # once, only the live half is re-DMA'd (no per-block memset).
                qz_bufs = [[p2.tile([128, TPW], BF16, name=f"qz{p}{j}",
                                    tag=f"qz{p}{j}") for j in range(2)]
                           for p in range(2)]
                for p in range(2):
                    for j in range(2):
                        nc.vector.memset(qz_bufs[p][j][(1 - p) * 64:(2 - p) * 64, :], 0.0)
                qz_cnt = [0, 0]

                def fetch_qz(h, tp):
                    ti, ro = h // 2, (h % 2) * 64
                    tps = slice(tp * TPW, (tp + 1) * TPW)
                    p = h % 2
                    qz = qz_bufs[p][qz_cnt[p] % 2]
                    qz_cnt[p] += 1
                    nc.sync.dma_start(qz[ro:ro + 64, :], qT[ti][ro:ro + 64, tps])
                    return qz

                def outproj(tt):
                    py = psp.tile([128, D], F32, name="slot", tag="slot", bufs=2)
                    for ci in range(NDT):
                        for nh in range(2):
                            nc.tensor.matmul(
                                py[:, nh * 512:(nh + 1) * 512],
                                ctx[ci][:, tt * 128:(tt + 1) * 128],
                                wo_sb[ci][:, nh * 512:(nh + 1) * 512],
                                start=(ci == 0), stop=(ci == NDT - 1))
                    yt = p2.tile([128, D], F32, name="y_t", tag="y_t", bufs=2)
                    nc.vector.tensor_add(yt[:], py[:], bo_sb[:])
                    nc.scalar.dma_start(y[tt * 128:(tt + 1) * 128, :], yt[:])

                # blocks run tp-major so the low-t half of ctx completes
                # mid-kernel; out-projection for those t-tiles interleaves
                # there, keeping the PE warm through the final norms.
                qz_next = fetch_qz(0, 0)
                for bi, (h, tp) in enumerate(blocks):
                    ti, ro = h // 2, (h % 2) * 64
                    qz = qz_next
                    if bi + 1 < len(blocks):
                        qz_next = fetch_qz(*blocks[bi + 1])
                    if bi >= 9:
                        outproj(bi - 9)  # ctx[:, :1024] complete by block 9
                    e_list = []
                    pc = psp.tile([65, TPW], F32, name="ctx_ps", tag="ctx_ps", bufs=2)
                    for si in range(NSI):
                        if prev is not None:
                            emit_pv(prev, si)
                        s0 = si * 128
                        kst = kT[ti][:, s0:s0 + 128]
                        ps = psp.tile([128, TPW], F32, name="slot", tag="slot", bufs=2)
                        nc.tensor.matmul(ps[:, 0:512], kst,
                                         qz[:, 0:512], start=True, stop=True)
                        nc.tensor.matmul(ps[:, 512:1024], kst,
                                         qz[:, 512:1024], start=True, stop=True)
                        et = p2.tile([128, TPW], BF16, name="e_t", tag="e_t", bufs=17)
                        if si in DVE_EXP_SI:
                            with nc.allow_low_precision(reason="schraudolph exp"):
                                nc.vector.tensor_scalar(
                                    et[:].bitcast(mybir.dt.int16), ps[:],
                                    SCH_A, SCH_B,
                                    mybir.AluOpType.mult, mybir.AluOpType.add)
                        else:
                            nc.scalar.activation(et[:], ps[:], Exp, scale=SCALE)
                        e_list.append(et)
                    if prev is not None:
                        emit_norm(prev)
                    prev = (h, tp, e_list, pc)
                for si in range(NSI):
                    emit_pv(prev, si)
                emit_norm(prev)

                # ---------------- Phase 3: out-projection (rest) ----------
                for tt in range(7, NTT):
                    outproj(tt)

    nc.compile()
    return nc


_PROGRAM = None


def _get_program():
    global _PROGRAM
    if _PROGRAM is None:
        _PROGRAM = build_program()
    return _PROGRAM


def _bf16(a):
    return np.ascontiguousarray(a.astype(ml_dtypes.bfloat16))


def make_in_maps(x, w_qkv, b_qkv, w_out, b_out):
    x = np.asarray(x, dtype=np.float32)
    w_qkv = np.asarray(w_qkv, dtype=np.float32)
    b_qkv = np.asarray(b_qkv, dtype=np.float32)
    w_out = np.asarray(w_out, dtype=np.float32)
    b_out = np.asarray(b_out, dtype=np.float32)

    in_maps = []
    for core in range(NCORES):
        b, g = divmod(core, G)
        gs = slice(g * HD, (g + 1) * HD)
        bo_part = b_out if g == 0 else np.zeros_like(b_out)
        in_maps.append({
            "xt": _bf16(x[b].T),
            "wq": _bf16(w_qkv[:, 0 * D:1 * D][:, gs]),
            "wk": _bf16(w_qkv[:, 1 * D:2 * D][:, gs]),
            "wv": _bf16(w_qkv[:, 2 * D:3 * D][:, gs]),
            "bq": np.ascontiguousarray(b_qkv[0 * D:1 * D][gs].reshape(HD, 1)),
            "bk": np.ascontiguousarray(b_qkv[1 * D:2 * D][gs].reshape(HD, 1)),
            "bv": np.ascontiguousarray(
                np.broadcast_to(b_qkv[2 * D:3 * D][gs], (128, HD))),
            "wo": _bf16(w_out[gs, :]),
            "bo": np.ascontiguousarray(np.broadcast_to(bo_part, (128, D))),
        })
    return in_maps


def run(inputs, trace=False, tmpdir=None):
    nc = _get_program()
    in_maps = make_in_maps(**inputs)
    res = run_bass_kernel_spmd(nc, in_maps, list(range(NCORES)),
                               trace=trace, tmpdir=tmpdir)
    parts = [np.asarray(res.results[c]["y"]) for c in range(NCORES)]
    out = np.empty((B, T, D), dtype=np.float32)
    for b in range(B):
        out[b] = parts[b * G + 0] + parts[b * G + 1]
    return out, res


def kernel(**inputs) -> np.ndarray:
    out, _ = run(inputs, trace=False)
    return out


# revision 20
# speedup vs baseline: 1.0501x; 1.0501x over previous
"""Multi-head self-attention (B=4, T=2048, D=1024, H=16) on 8 trn2 cores.

Sharding: core = b * 2 + g  (b = batch 0..3, g = head-group 0..1, 8 heads each).
Each core computes, for its (batch, 8-head group):
  Q^T,K^T [hd=512, T] and V [T, hd] from x^T (host-pretransposed, bf16),
  per head: E = exp((K^T_h)^T @ Q^T_h / 8)  in [s, t] layout,
  ctx^T/denominator via PV matmul with a ones-row appended to V,
  out-projection partial y_part = ctx^T.T @ w_out[g rows] + b_out (g==0 only).
Host sums the two head-group partials per batch.

All matmuls run in bf16 (1 row/cycle on PE; same rate as f32r but half
the SBUF/DMA); attention tiles are 1024 wide to halve instruction count.
"""

import numpy as np
import ml_dtypes
import concourse.bass as bass
import concourse.bacc as bacc
import concourse.mybir as mybir
import concourse.tile as tile
from concourse.bass_utils import run_bass_kernel_spmd

B, T, D = 4, 2048, 1024
H, DK = 16, 64
G = 2                 # head groups
HPG = H // G          # heads per core = 8
HD = HPG * DK         # 512
NCORES = B * G        # 8
SCALE = 1.0 / float(np.sqrt(DK))

F32 = mybir.dt.float32
BF16 = mybir.dt.bfloat16
Ident = mybir.ActivationFunctionType.Identity
Exp = mybir.ActivationFunctionType.Exp

NCC = D // 128        # 8 contraction chunks
NDT = HD // 128       # 4 d-tiles of Q/K per core
NTT = T // 128        # 16 t-tiles
NSI = T // 128        # 16 s-tiles
NTP = 2               # t-halves of 1024
TPW = T // NTP        # 1024
VW = HPG * (DK + 1)   # 520: V padded with a ones column per head

# knobs
DVE_EXP_SI = {2, 5, 8, 11, 13, 15}   # si whose exp runs on DVE (Schraudolph)

# Schraudolph constants for bf16-bits exp: i16 = round(score*A + Bc)
SCH_A = SCALE * 128.0 / float(np.log(2.0))
SCH_B = 127.0 * 128.0 - 8.0


def build_program():
    nc = bacc.Bacc("TRN2", target_bir_lowering=False, debug=False)

    xt = nc.dram_tensor("xt", [D, T], BF16, kind="ExternalInput").ap()
    wq = nc.dram_tensor("wq", [D, HD], BF16, kind="ExternalInput").ap()
    wk = nc.dram_tensor("wk", [D, HD], BF16, kind="ExternalInput").ap()
    wv = nc.dram_tensor("wv", [D, HD], BF16, kind="ExternalInput").ap()
    bq = nc.dram_tensor("bq", [HD, 1], F32, kind="ExternalInput").ap()
    bk = nc.dram_tensor("bk", [HD, 1], F32, kind="ExternalInput").ap()
    bv = nc.dram_tensor("bv", [128, HD], F32, kind="ExternalInput").ap()
    wo = nc.dram_tensor("wo", [HD, D], BF16, kind="ExternalInput").ap()
    bo = nc.dram_tensor("bo", [128, D], F32, kind="ExternalInput").ap()
    y = nc.dram_tensor("y", [T, D], F32, kind="ExternalOutput").ap()

    with tile.TileContext(nc) as tc:
        with tc.tile_pool(name="persist", bufs=1) as pp, \
             tc.tile_pool(name="ps", bufs=1, space="PSUM") as psp:
            # persistent QKV outputs (bf16)
            qT = [pp.tile([128, T], BF16, name=f"qT{i}", tag=f"qT{i}") for i in range(NDT)]
            kT = [pp.tile([128, T], BF16, name=f"kT{i}", tag=f"kT{i}") for i in range(NDT)]
            vv = [pp.tile([128, VW], BF16, name=f"v{i}", tag=f"v{i}") for i in range(NTT)]
            ctx = [pp.tile([128, T], BF16, name=f"ctx{i}", tag=f"ctx{i}")
                   for i in range(NDT)]

            # out-proj weights, prefetched early (used in phase 3)
            wo_sb = [pp.tile([128, D], BF16, name=f"wo{c}", tag=f"wo{c}")
                     for c in range(NDT)]
            bo_sb = pp.tile([128, D], F32, name="bo_sb", tag="bo_sb")

            # attention working tiles live in p2 (kept open across
            # phases so kb/qd prefetch for the first block can be emitted
            # during phase 1); p1 closes after phase 1 to free xt/w SBUF.
            with tc.tile_pool(name="p2", bufs=1) as p2, \
                 tc.tile_pool(name="dpool", bufs=1, space="DRAM") as dpool:
                # ---------------- Phase 1: QKV projections ----------------
                with tc.tile_pool(name="p1", bufs=1) as p1:
                    ones_f32 = p1.tile([128, 64], F32, name="ones_f32", tag="ones_f32")
                    nc.vector.memset(ones_f32[:], 1.0)
                    # DMA issue order tuned so the first chains can start
                    # ASAP: interleave x half0 chunks with wk (K goes first).
                    xt_half = [[None] * NCC, [None] * NCC]
                    wk_sb = [p1.tile([128, HD], BF16, name=f"wk{c}", tag=f"wk{c}") for c in range(NCC)]
                    wq_sb = [p1.tile([128, HD], BF16, name=f"wq{c}", tag=f"wq{c}") for c in range(NCC)]
                    for c in range(NCC):
                        xh = p1.tile([128, TPW], BF16, name=f"xt_h0_{c}", tag="xt", bufs=12)
                        nc.sync.dma_start(xh[:], xt[c * 128:(c + 1) * 128, 0:TPW])
                        xt_half[0][c] = xh
                        nc.sync.dma_start(wk_sb[c][:], wk[c * 128:(c + 1) * 128, :])
                    bk_sb = [p1.tile([128, 1], F32, name=f"bk{i}", tag=f"bk{i}") for i in range(NDT)]
                    for i in range(NDT):
                        nc.sync.dma_start(bk_sb[i][:], bk[i * 128:(i + 1) * 128, :])
                    for c in range(NCC):
                        nc.sync.dma_start(wq_sb[c][:], wq[c * 128:(c + 1) * 128, :])
                    bq_sb = [p1.tile([128, 1], F32, name=f"bq{i}", tag=f"bq{i}") for i in range(NDT)]
                    for i in range(NDT):
                        nc.sync.dma_start(bq_sb[i][:], bq[i * 128:(i + 1) * 128, :])
                    for c in range(NCC):
                        xh = p1.tile([128, TPW], BF16, name=f"xt_h1_{c}", tag="xt", bufs=12)
                        nc.sync.dma_start(xh[:], xt[c * 128:(c + 1) * 128, TPW:T])
                        xt_half[1][c] = xh
                    wv_sb = [p1.tile([128, HD], BF16, name=f"wv{c}", tag=f"wv{c}") for c in range(NCC)]
                    for c in range(NCC):
                        nc.sync.dma_start(wv_sb[c][:], wv[c * 128:(c + 1) * 128, :])
                    bv_sb = p1.tile([128, HD], F32, name="bv_sb", tag="bv_sb")
                    nc.sync.dma_start(bv_sb[:], bv[:])
                    for c in range(NDT):
                        nc.sync.dma_start(wo_sb[c][:], wo[c * 128:(c + 1) * 128, :])
                    nc.sync.dma_start(bo_sb[:], bo[:])

                    def qk_chain(w_sb, b_sb, out_t, dt_i, half):
                        # psum[d,t] += w[c,d]^T.T @ xt[c,t]  (two N=512 chains
                        # into one [128,1024] slot, drained by one ACT)
                        ps = psp.tile([128, TPW], F32, name="slot", tag="slot", bufs=2)
                        for tci in range(2):
                            for c in range(NCC):
                                nc.tensor.matmul(
                                    ps[:, tci * 512:(tci + 1) * 512],
                                    w_sb[c][:, dt_i * 128:(dt_i + 1) * 128],
                                    xt_half[half][c][:, tci * 512:(tci + 1) * 512],
                                    start=(c == 0), stop=(c == NCC - 1))
                        nc.scalar.activation(
                            out_t[dt_i][:, half * TPW:(half + 1) * TPW],
                            ps[:], Ident, bias=b_sb[dt_i][:])

                    def v_pair(half, tp):
                        # two V t-tiles share one [128,1024] psum slot
                        ps = psp.tile([128, TPW], F32, name="slot", tag="slot", bufs=2)
                        for j in range(2):
                            for c in range(NCC):
                                nc.tensor.matmul(
                                    ps[:, j * 512:(j + 1) * 512],
                                    xt_half[half][c][:, (tp * 2 + j) * 128:(tp * 2 + j + 1) * 128],
                                    wv_sb[c][:],
                                    start=(c == 0), stop=(c == NCC - 1))
                        for j in range(2):
                            tt = half * 8 + tp * 2 + j
                            vt = vv[tt]
                            v3 = vt[:].rearrange("p (h e) -> p h e", e=DK + 1)
                            with nc.allow_low_precision(reason="bf16 for PE"):
                                nc.vector.tensor_add(
                                    v3[:, :, 0:DK],
                                    ps[:, j * 512:(j + 1) * 512].rearrange(
                                        "p (h e) -> p h e", e=DK),
                                    bv_sb[:].rearrange("p (h e) -> p h e", e=DK))
                                nc.vector.tensor_copy(
                                    v3[:, :, DK:DK + 1],
                                    ones_f32[:, 0:HPG].rearrange("p (h e) -> p h e", e=1))

                    # half 0: K first (kb gate), Q dt0 (qd gate), V, rest of Q
                    for dt_i in range(NDT):
                        qk_chain(wk_sb, bk_sb, kT, dt_i, 0)
                    qk_chain(wq_sb, bq_sb, qT, 0, 0)
                    for tp in range(4):
                        v_pair(0, tp)
                    for dt_i in range(1, NDT):
                        qk_chain(wq_sb, bq_sb, qT, dt_i, 0)
                    # half 1: K dt0 + Q dt0 first, then prefetch h0's kb/qd so
                    # the first attention block can start right at phase end.
                    qk_chain(wk_sb, bk_sb, kT, 0, 1)
                    qk_chain(wq_sb, bq_sb, qT, 0, 1)
                    for dt_i in range(1, NDT):
                        qk_chain(wk_sb, bk_sb, kT, dt_i, 1)
                    for tp in range(4):
                        v_pair(1, tp)
                    for dt_i in range(1, NDT):
                        qk_chain(wq_sb, bq_sb, qT, dt_i, 1)

                # ---------------- Phase 2: attention ----------------
                # blocks: h outer, tp (t-half of 1024) inner; kb cached per h.
                blocks = [(h, tp) for h in range(HPG) for tp in range(NTP)]
                prev = None  # (h, tp, e_list, pc)

                def emit_pv(prev, si):
                    h, tp, e_list, pc = prev
                    vsl = vv[si][:, h * (DK + 1):(h + 1) * (DK + 1)]
                    for half in range(2):
                        nc.tensor.matmul(
                            pc[:, half * 512:(half + 1) * 512],
                            vsl, e_list[si][:, half * 512:(half + 1) * 512],
                            start=(si == 0), stop=(si == NSI - 1))

                def emit_norm(prev):
                    # ctx[d,t] = pc[d,t] / pc[64,t]. The [1,1024] denominator
                    # row is bounced through DRAM to spread it over 64
                    # partitions: DVE reciprocal costs 8 cyc/elem/lane, so a
                    # single-partition reciprocal would be 8.5us.
                    h, tp, e_list, pc = prev
                    ti, ro = h // 2, (h % 2) * 64
                    tps = slice(tp * TPW, (tp + 1) * TPW)
                    dsb = p2.tile([1, TPW], F32, name="dn_row", tag="dn_row", bufs=1)
                    nc.scalar.copy(dsb[:], pc[64:65, :])
                    scr1 = dpool.tile([1, TPW], F32, name="dscr", tag="dscr", bufs=2)
                    nc.gpsimd.dma_start(scr1[:], dsb[:])
                    d64 = p2.tile([64, 16], F32, name="d64", tag="d64", bufs=2)
                    nc.gpsimd.dma_start(
                        d64[:], scr1[:].rearrange("o (p f) -> (o p) f", p=64))
                    r64 = p2.tile([64, 16], F32, name="r64", tag="r64", bufs=2)
                    nc.vector.reciprocal(r64[:], d64[:])
                    scr2 = dpool.tile([1, TPW], F32, name="rscr", tag="rscr", bufs=2)
                    nc.gpsimd.dma_start(
                        scr2[:].rearrange("o (p f) -> (o p) f", p=64), r64[:])
                    r = p2.tile([1, TPW], F32, name="r_t", tag="r_t", bufs=1)
                    nc.gpsimd.dma_start(r[:], scr2[:])
                    rb = p2.tile([64, TPW], F32, name="rb_t", tag="rb_t", bufs=2)
                    nc.gpsimd.partition_broadcast(rb[:], r[:])
                    with nc.allow_low_precision(reason="bf16 for PE"):
                        nc.vector.tensor_mul(
                            ctx[ti][ro:ro + 64, tps], pc[0:64, :], rb[:])

                # Scores use the full [128, s-block] kT slice as the
                # stationary (both heads of the pair; K=128 keeps the HAM
                # clock gate open) against a moving Q whose other-head rows
                # are zeroed, so the off-head contribution vanishes. Two
                # persistent buffers per parity: the zero half is written
                # once, only the live half is re-DMA'd (no per-block memset).
                qz_bufs = [[p2.tile([128, TPW], BF16, name=f"qz{p}{j}",
                                    tag=f"qz{p}{j}") for j in range(2)]
                           for p in range(2)]
                for p in range(2):
                    for j in range(2):
                        nc.vector.memset(qz_bufs[p][j][(1 - p) * 64:(2 - p) * 64, :], 0.0)
                qz_cnt = [0, 0]

                def fetch_qz(h, tp):
                    ti, ro = h // 2, (h % 2) * 64
                    tps = slice(tp * TPW, (tp + 1) * TPW)
                    p = h % 2
                    qz = qz_bufs[p][qz_cnt[p] % 2]
                    qz_cnt[p] += 1
                    nc.sync.dma_start(qz[ro:ro + 64, :], qT[ti][ro:ro + 64, tps])
                    return qz

                def outproj(tt):
                    py = psp.tile([128, D], F32, name="slot", tag="slot", bufs=2)
                    for ci in range(NDT):
                        for nh in range(2):
                            nc.tensor.matmul(
                                py[:, nh * 512:(nh + 1) * 512],
                                ctx[ci][:, tt * 128:(tt + 1) * 128],
                                wo_sb[ci][:, nh * 512:(nh + 1) * 512],
                                start=(ci == 0), stop=(ci == NDT - 1))
                    yt = p2.tile([128, D], F32, name="y_t", tag="y_t", bufs=2)
                    nc.vector.tensor_add(yt[:], py[:], bo_sb[:])
                    nc.scalar.dma_start(y[tt * 128:(tt + 1) * 128, :], yt[:])

                # blocks run tp-major so the low-t half of ctx completes
                # mid-kernel; out-projection for those t-tiles interleaves
                # there, keeping the PE warm through the final norms.
                qz_next = fetch_qz(0, 0)
                for bi, (h, tp) in enumerate(blocks):
                    ti, ro = h // 2, (h % 2) * 64
                    qz = qz_next
                    if bi + 1 < len(blocks):
                        qz_next = fetch_qz(*blocks[bi + 1])
                    e_list = []
                    pc = psp.tile([65, TPW], F32, name="ctx_ps", tag="ctx_ps", bufs=2)
                    for si in range(NSI):
                        if prev is not None:
                            emit_pv(prev, si)
                        s0 = si * 128
                        kst = kT[ti][:, s0:s0 + 128]
                        ps = psp.tile([128, TPW], F32, name="slot", tag="slot", bufs=2)
                        nc.tensor.matmul(ps[:, 0:512], kst,
                                         qz[:, 0:512], start=True, stop=True)
                        nc.tensor.matmul(ps[:, 512:1024], kst,
                                         qz[:, 512:1024], start=True, stop=True)
                        et = p2.tile([128, TPW], BF16, name="e_t", tag="e_t", bufs=17)
                        if si in DVE_EXP_SI:
                            with nc.allow_low_precision(reason="schraudolph exp"):
                                nc.vector.tensor_scalar(
                                    et[:].bitcast(mybir.dt.int16), ps[:],
                                    SCH_A, SCH_B,
                                    mybir.AluOpType.mult, mybir.AluOpType.add)
                        else:
                            nc.scalar.activation(et[:], ps[:], Exp, scale=SCALE)
                        e_list.append(et)
                    if prev is not None:
                        emit_norm(prev)
                    prev = (h, tp, e_list, pc)
                for si in range(NSI):
                    emit_pv(prev, si)
                emit_norm(prev)

                # ---------------- Phase 3: out-projection (rest) ----------
                for tt in range(NTT):
                    outproj(tt)

    nc.compile()
    return nc


_PROGRAM = None


def _get_program():
    global _PROGRAM
    if _PROGRAM is None:
        _PROGRAM = build_program()
    return _PROGRAM


def _bf16(a):
    return np.ascontiguousarray(a.astype(ml_dtypes.bfloat16))


def make_in_maps(x, w_qkv, b_qkv, w_out, b_out):
    x = np.asarray(x, dtype=np.float32)
    w_qkv = np.asarray(w_qkv, dtype=np.float32)
    b_qkv = np.asarray(b_qkv, dtype=np.float32)
    w_out = np.asarray(w_out, dtype=np.float32)
    b_out = np.asarray(b_out, dtype=np.float32)

    in_maps = []
    for core in range(NCORES):
        b, g = divmod(core, G)
        gs = slice(g * HD, (g + 1) * HD)
        bo_part = b_out if g == 0 else np.zeros_like(b_out)
        in_maps.append({
            "xt": _bf16(x[b].T),
            "wq": _bf16(w_qkv[:, 0 * D:1 * D][:, gs]),
            "wk": _bf16(w_qkv[:, 1 * D:2 * D][:, gs]),
            "wv": _bf16(w_qkv[:, 2 * D:3 * D][:, gs]),
            "bq": np.ascontiguousarray(b_qkv[0 * D:1 * D][gs].reshape(HD, 1)),
            "bk": np.ascontiguousarray(b_qkv[1 * D:2 * D][gs].reshape(HD, 1)),
            "bv": np.ascontiguousarray(
                np.broadcast_to(b_qkv[2 * D:3 * D][gs], (128, HD))),
            "wo": _bf16(w_out[gs, :]),
            "bo": np.ascontiguousarray(np.broadcast_to(bo_part, (128, D))),
        })
    return in_maps


def run(inputs, trace=False, tmpdir=None):
    nc = _get_program()
    in_maps = make_in_maps(**inputs)
    res = run_bass_kernel_spmd(nc, in_maps, list(range(NCORES)),
                               trace=trace, tmpdir=tmpdir)
    parts = [np.asarray(res.results[c]["y"]) for c in range(NCORES)]
    out = np.empty((B, T, D), dtype=np.float32)
    for b in range(B):
        out[b] = parts[b * G + 0] + parts[b * G + 1]
    return out, res


def kernel(**inputs) -> np.ndarray:
    out, _ = run(inputs, trace=False)
    return out
